# revision 25
# baseline (speedup 1.0000x reference)
"""ClusteredAttention Trainium2 kernel.

Strategy
--------
The reference masks attention to same-cluster pairs: with 32 clusters over
L=4096 tokens each row attends to ~128 keys, so attention is block-diagonal
after sorting tokens by cluster on the host.  B*N_CLUSTERS = 128 pairs are
dealt to 8 cores (16 pairs/core, rank-sorted so every slot has a
near-uniform shape, giving one SPMD program for all cores).

Per (batch, cluster) on device, with Qt/Kt = [D, n] transposed layouts:
  S^T[key, query] = sum_d Kt[d,key] * Qt[d,query]      (accumulating matmuls)
  E = exp(SCALE*S^T + bias)    bias = -100 on padded key rows -> E = 0
  O^T[d, query]   = sum_key V[key,d] * E[key,query]    (d chunks of 128)
  den[1, query]   = sum_key E[key,query]               (ones-vector matmul)
Host divides O^T by den, transposes, and scatters rows back.

The problem is DMA-byte-bound (~310 GB/s/core effective, measured), so the
layout ships minimum bytes: qcols is the exact group-max cluster size
(rounded to 2), V rows 128..qc of >128-token clusters live in a compact
32-partition side tensor instead of a padded 128-row block, and the O^T
output layout (queries on the free dim) writes exactly DC*qc columns per
slot -- no padded 128-row output block for a <=32-row query sliver.
int8 shipping was tried and rejected: the int8->bf16 upconvert costs more
ACT/DVE engine time than the saved bytes (PE matmul takes no int8).

All input DMAs are prefetched at body start on the SP HWDGE queue; output
DMAs ride the Activation queue so a compute-gated output descriptor never
blocks input prefetch (HWDGE descriptors drain in order per engine).
Scores for slot j+1 are emitted before O^T of slot j so exp latency hides
under PE work and the PE stays continuously busy (p-state ramp).  The two
PSUM->SBUF output copies per slot alternate ACT/DVE.

Numerics: Q,K,V and E are bf16; every accumulation (scores, O, den) is f32
on the PE/PSUM path; the same bf16 E feeds numerator and denominator so
rounding largely cancels.
"""

import math

import numpy as np
import ml_dtypes

import concourse.bass as bass
import concourse.mybir as mybir
import concourse.tile as tile
from concourse import bacc
from concourse.bass_utils import run_bass_kernel_spmd

B, L, D, NC = 4, 4096, 512, 32
N_CORES = 8
SLOTS = (B * NC) // N_CORES  # 16 (batch, cluster) pairs per core
GROUP = 4  # slots per DMA group
P = 128
DC = D // P  # 4 contraction / output chunks
SCALE = 1.0 / math.sqrt(D)
NEG = -100.0  # exp(-100) underflows to 0 in bf16
BF16 = ml_dtypes.bfloat16

LAST_RESULT = None  # BassKernelResults of the most recent run (for test.py)


def _ceil(a, b):
    return -(-a // b)


def _plan(label_arr):
    """Sort the 128 (batch, cluster) pairs by size, deal to cores/slots, and
    lay out per-group input blobs / output regions.

    slots[j]: qcols (exact group-max n, even), nkb, kw1, group, qoff (Q),
              koff (K), voff (V kb0), v2off (V kb1), ooff (out), bcol
    groups[g]: slots, free (blob cols), ofree, v2cols
    assign[core][j] = (b, c, idx, n)
    """
    pairs = []
    for b in range(B):
        lab = label_arr[b]
        for c in range(NC):
            idx = np.nonzero(lab == c)[0]
            pairs.append((b, c, idx, len(idx)))
    pairs.sort(key=lambda t: -t[3])

    slots = []
    assign = [[None] * SLOTS for _ in range(N_CORES)]
    for j in range(SLOTS):
        group = pairs[j * N_CORES : (j + 1) * N_CORES]
        maxn = max(max(t[3] for t in group), 8)
        qc = _ceil(maxn, 2) * 2
        nkb = _ceil(qc, P)
        assert nkb <= 2, "cluster larger than 256 tokens unsupported"
        slots.append(dict(qcols=qc, nkb=nkb, kw1=qc - P if nkb == 2 else 0))
        for core in range(N_CORES):
            assign[core][j] = group[core]

    p2 = max([_ceil(s["kw1"], 8) * 8 for s in slots if s["nkb"] == 2],
             default=8)

    bcol = 0
    for s in slots:
        s["bcol"] = bcol
        bcol += s["nkb"]
    ncol = bcol

    groups = []
    order = list(range(SLOTS - 1, -1, -1))  # smallest slots first: quick start
    for g in range(_ceil(SLOTS, GROUP)):
        js = order[g * GROUP : (g + 1) * GROUP]
        off = ooff = v2off = 0
        for j in js:
            s = slots[j]
            s["group"] = g
            s["qoff"] = off
            off += DC * s["qcols"]
            s["koff"] = off
            off += DC * s["qcols"]
            s["voff"] = off
            off += D
            s["ooff"] = ooff
            ooff += DC * s["qcols"]
            if s["nkb"] == 2:
                s["v2off"] = v2off
                v2off += D
        groups.append(dict(slots=js, free=off, ofree=ooff, v2cols=v2off))
    meta = dict(p2=p2, ncol=ncol, qmax=max(s["qcols"] for s in slots))
    return slots, groups, assign, meta


def _build_program(slots, groups, meta, loop_n=None, mode="full"):
    """Build the SPMD Bass program (identical for all 8 cores).

    loop_n: bench-only -- repeat the whole body loop_n times on-device via
    tc.For_i so steady-state HW time per iteration can be measured.
    mode: bench-only -- "dma" strips compute, "compute" strips DMA;
    "noden"/"nocopy"/"nodep" are compute ablations.
    """
    cmodes = ("compute", "noden", "nocopy", "nodep")
    nc = bacc.Bacc("TRN2", target_bir_lowering=False, debug=False,
                   num_devices=N_CORES)
    p2, ncol, qmax = meta["p2"], meta["ncol"], meta["qmax"]

    in_d = [nc.dram_tensor(f"in{g}", (P, gr["free"]), mybir.dt.bfloat16,
                           kind="ExternalInput") for g, gr in enumerate(groups)]
    v2_d = [nc.dram_tensor(f"v2_{g}", (p2, gr["v2cols"]), mybir.dt.bfloat16,
                           kind="ExternalInput") if gr["v2cols"] else None
            for g, gr in enumerate(groups)]
    o_d = [nc.dram_tensor(f"o{g}", (P, gr["ofree"]), mybir.dt.bfloat16,
                          kind="ExternalOutput") for g, gr in enumerate(groups)]
    bias_d = nc.dram_tensor("bias", (P, ncol), mybir.dt.float32,
                            kind="ExternalInput")
    den_d = nc.dram_tensor("den", (1, SLOTS * qmax), mybir.dt.float32,
                           kind="ExternalOutput")

    with tile.TileContext(nc) as tc:
        with (
            tc.tile_pool(name="persist", bufs=1) as persist,
            tc.tile_pool(name="exp", bufs=6) as exp_pool,
            tc.tile_pool(name="ps", bufs=3, space="PSUM") as ps_pool,
            tc.tile_pool(name="ps2", bufs=2, space="PSUM") as ps2_pool,
            tc.tile_pool(name="po", bufs=2, space="PSUM") as po_pool,
            tc.tile_pool(name="pd", bufs=1, space="PSUM") as pd_pool,
        ):
            bias_ld = persist.tile([P, ncol], mybir.dt.float32, tag="bias_ld")
            nc.sync.dma_start(bias_ld, bias_d[:, :])
            # Relay through ScalarE so the per-slot exp activations depend on
            # bias via same-engine program order instead of an extra sem wait.
            bias_sb = persist.tile([P, ncol], mybir.dt.float32, tag="bias")
            nc.scalar.copy(bias_sb, bias_ld)
            ones = persist.tile([P, 1], mybir.dt.bfloat16, tag="ones")
            nc.vector.memset(ones, 1.0)
            dummy = persist.tile([P, qmax], mybir.dt.bfloat16, tag="dummy")
            nc.vector.memset(dummy, 0.001)
            cnt = None
            if loop_n:
                cnt = persist.tile([P, 16], mybir.dt.float32, tag="cnt")
                nc.vector.memset(cnt, 0.0)

            # Touch every scores-PSUM buffer once so the first-use exp (which
            # reads unwritten tail rows of slots with <128 keys) sees finite
            # values.  After that, stale tail rows are old score values and
            # exp(s*SCALE - 100) underflows to 0 anyway.
            for _ in range(3):
                ps0 = ps_pool.tile([P, qmax], mybir.dt.float32, tag="ps")
                nc.vector.memset(ps0, 0.0)
            for _ in range(2):
                ps20 = ps2_pool.tile([p2, qmax], mybir.dt.float32, tag="ps2")
                nc.vector.memset(ps20, 0.0)

            blob, v2blob, ostg = {}, {}, {}
            for g, gr in enumerate(groups):
                bt = persist.tile([P, gr["free"]], mybir.dt.bfloat16,
                                  tag=f"blob{g}")
                og = persist.tile([P, gr["ofree"]], mybir.dt.bfloat16,
                                  tag=f"ostg{g}")
                blob[g], ostg[g] = bt, og
                if gr["v2cols"]:
                    v2b = persist.tile([p2, gr["v2cols"]], mybir.dt.bfloat16,
                                       tag=f"v2blob{g}")
                    v2blob[g] = v2b
                if mode in cmodes:
                    nc.vector.memset(bt, 0.001)
                    if gr["v2cols"]:
                        nc.vector.memset(v2blob[g], 0.001)
                if mode == "dma":
                    nc.vector.memset(og, 0.0)

            def scores_exp(j):
                """Emit scores matmuls + exp for slot j."""
                s = slots[j]
                g, qc, nkb = s["group"], s["qcols"], s["nkb"]
                qt = blob[g][:, s["qoff"] : s["qoff"] + DC * qc].rearrange(
                    "p (dc m) -> p dc m", dc=DC)
                kt = blob[g][:, s["koff"] : s["koff"] + DC * qc].rearrange(
                    "p (dc m) -> p dc m", dc=DC)
                exs = []
                for kb in range(nkb):
                    kw = min(P, qc) if kb == 0 else s["kw1"]
                    pp = P if kb == 0 else p2
                    pool_, tag = (ps_pool, "ps") if kb == 0 else (ps2_pool,
                                                                  "ps2")
                    ps = pool_.tile([pp, qc], mybir.dt.float32, tag=tag)
                    for dc in range(DC):
                        nc.tensor.matmul(
                            ps[:kw],
                            lhsT=kt[:, dc, kb * P : kb * P + kw],
                            rhs=qt[:, dc, :],
                            start=(dc == 0),
                            stop=(dc == DC - 1),
                        )
                    ex = exp_pool.tile([pp, qc], mybir.dt.bfloat16,
                                       tag=("ex" if kb == 0 else "ex2"))
                    bc = s["bcol"] + kb
                    nc.scalar.activation(
                        ex,
                        ps,
                        mybir.ActivationFunctionType.Exp,
                        scale=SCALE,
                        bias=bias_sb[:pp, bc : bc + 1],
                    )
                    exs.append(ex)
                return exs

            def o_den(j, pos, exs):
                """Emit O^T matmuls + PSUM->SBUF copies + den for slot j."""
                s = slots[j]
                g, qc, nkb = s["group"], s["qcols"], s["nkb"]
                vs = [blob[g][:, s["voff"] : s["voff"] + D]]
                if nkb == 2:
                    vs.append(v2blob[g][:, s["v2off"] : s["v2off"] + D])
                if mode == "nodep":
                    exs = [dummy[: (P if kb == 0 else p2), :qc]
                           for kb in range(nkb)]
                for pair in range(DC // 2):
                    po = po_pool.tile([P, 2, qc], mybir.dt.float32, tag="po")
                    for i in range(2):
                        dc = 2 * pair + i
                        for kb in range(nkb):
                            nc.tensor.matmul(
                                po[:, i, :],
                                lhsT=vs[kb][:, dc * P : (dc + 1) * P],
                                rhs=exs[kb],
                                start=(kb == 0),
                                stop=(kb == nkb - 1),
                            )
                    if mode != "nocopy":
                        dst = ostg[g][:, s["ooff"] + 2 * pair * qc :
                                      s["ooff"] + 2 * (pair + 1) * qc]
                        src = po[:, 0:2, 0:qc].rearrange("p a b -> p (a b)")
                        if pair % 2 == 0:
                            nc.vector.tensor_copy(dst, src)
                        else:
                            nc.scalar.copy(dst, src)

                if mode not in ("noden", "nocopy"):
                    pd = pd_pool.tile([1, qc], mybir.dt.float32, tag="pd")
                    for kb in range(nkb):
                        pp = P if kb == 0 else p2
                        nc.tensor.matmul(
                            pd,
                            lhsT=ones[:pp],
                            rhs=exs[kb],
                            start=(kb == 0),
                            stop=(kb == nkb - 1),
                        )
                    nc.vector.tensor_copy(
                        den_sb[:, pos * qmax : pos * qmax + qc], pd)

            def emit_body():
                nonlocal den_sb
                den_sb = persist.tile([1, SLOTS * qmax], mybir.dt.float32,
                                      tag="den_sb")
                # prefetch every input group up front on the SP queue
                if mode not in cmodes:
                    for g, gr in enumerate(groups):
                        nc.sync.dma_start(blob[g], in_d[g][:, :])
                        if gr["v2cols"]:
                            nc.sync.dma_start(v2blob[g], v2_d[g][:, :])

                if mode != "dma":
                    flat = [j for g, gr in enumerate(groups)
                            for j in gr["slots"]]
                    DEPTH = 2  # slots of scores emitted ahead of O^T
                    posof = {j: i for i, j in enumerate(flat)}

                    def finish(pj):
                        o_den(pj, posof[pj], pend.pop(pj))
                        pg = slots[pj]["group"]
                        if (mode not in cmodes
                                and pj == groups[pg]["slots"][-1]):
                            nc.gpsimd.dma_start(o_d[pg][:, :], ostg[pg])
                            c0 = (pg * GROUP) * qmax
                            c1 = c0 + len(groups[pg]["slots"]) * qmax
                            nc.gpsimd.dma_start(den_d[:, c0:c1],
                                                den_sb[:, c0:c1])

                    pend = {}
                    order = []
                    for j in flat:
                        pend[j] = scores_exp(j)
                        order.append(j)
                        if len(order) > DEPTH:
                            finish(order.pop(0))
                    while order:
                        finish(order.pop(0))
                else:
                    for g in range(len(groups)):
                        nc.gpsimd.dma_start(o_d[g][:, :], ostg[g])
                if cnt is not None:
                    nc.vector.tensor_scalar_add(cnt, cnt, 1.0)

            den_sb = None
            if loop_n:
                cnt_d = nc.dram_tensor("cnt", (P, 16), mybir.dt.float32,
                                       kind="ExternalOutput")
                with tc.For_i(0, loop_n, 1):
                    emit_body()
                nc.sync.dma_start(cnt_d[:, :], cnt)
            else:
                emit_body()

    nc.finalize()
    return nc


def _tchunk(x, qc):
    """[qc, D] row-major -> [P, DC*qc] partition-major d-chunked layout."""
    return x.T.reshape(DC, P, qc).transpose(1, 0, 2).reshape(P, DC * qc)


def _pack_inputs(query, key, value, slots, groups, assign, meta):
    p2, ncol = meta["p2"], meta["ncol"]
    in_maps = []
    for core in range(N_CORES):
        bias = np.zeros((P, ncol), np.float32)
        blobs = [np.zeros((P, gr["free"]), BF16) for gr in groups]
        v2blobs = [np.zeros((p2, gr["v2cols"]), BF16) for gr in groups]
        for j, s in enumerate(slots):
            qc, nkb, g = s["qcols"], s["nkb"], s["group"]
            b, c, idx, n = assign[core][j]
            qg = np.zeros((qc, D), np.float32)
            kg = np.zeros((qc, D), np.float32)
            vg = np.zeros((qc, D), np.float32)
            if n:
                qg[:n] = query[b, idx]
                kg[:n] = key[b, idx]
                vg[:n] = value[b, idx]
            blobs[g][:, s["qoff"] : s["qoff"] + DC * qc] = _tchunk(
                qg, qc).astype(BF16)
            blobs[g][:, s["koff"] : s["koff"] + DC * qc] = _tchunk(
                kg, qc).astype(BF16)
            vkb0 = np.zeros((P, D), np.float32)
            vkb0[: min(qc, P)] = vg[: min(qc, P)]
            blobs[g][:, s["voff"] : s["voff"] + D] = vkb0.astype(BF16)
            if nkb == 2:
                v2 = np.zeros((p2, D), np.float32)
                v2[: qc - P] = vg[P:qc]
                v2blobs[g][:, s["v2off"] : s["v2off"] + D] = v2.astype(BF16)
            for kb in range(nkb):
                pp = P if kb == 0 else p2
                nr = min(max(n - kb * P, 0), pp)
                bias[nr:pp, s["bcol"] + kb] = NEG

        m = {"bias": bias}
        for g, gr in enumerate(groups):
            m[f"in{g}"] = blobs[g]
            if gr["v2cols"]:
                m[f"v2_{g}"] = v2blobs[g]
        in_maps.append(m)
    return in_maps


def kernel(query, key, value, label_arr, trace=False):
    global LAST_RESULT
    query = np.asarray(query, dtype=np.float32)
    key = np.asarray(key, dtype=np.float32)
    value = np.asarray(value, dtype=np.float32)
    label_arr = np.asarray(label_arr)

    slots, groups, assign, meta = _plan(label_arr)
    nc = _build_program(slots, groups, meta)
    in_maps = _pack_inputs(query, key, value, slots, groups, assign, meta)

    res = run_bass_kernel_spmd(
        nc, in_maps, core_ids=list(range(N_CORES)), trace=trace
    )
    LAST_RESULT = res

    qmax = meta["qmax"]
    flat = [j for g, gr in enumerate(groups) for j in gr["slots"]]
    posof = {j: i for i, j in enumerate(flat)}
    out = np.zeros((B, L, D), np.float32)
    for core in range(N_CORES):
        r = res.results[core]
        den = np.asarray(r["den"]).reshape(SLOTS, qmax)
        for j, s in enumerate(slots):
            b, c, idx, n = assign[core][j]
            if n == 0:
                continue
            qc, g = s["qcols"], s["group"]
            og = np.asarray(r[f"o{g}"])  # [P, ofree] bf16
            blk = og[:, s["ooff"] : s["ooff"] + DC * qc].reshape(P, DC, qc)
            o_un = blk.transpose(2, 1, 0).reshape(qc, D)[:n].astype(np.float32)
            d = den[posof[j], :n].astype(np.float32)
            out[b, idx] = o_un / d[:, None]
    return out
